# revision 1
# baseline (speedup 1.0000x reference)
"""Trainium2 Bass kernel: 3-layer GAT (nn_GAT_62182536511748).

Strategy (8 NeuronCores, SPMD, fp16 pair-block gather):
  - Nodes sharded contiguously across cores (6250 valid/core, padded to
    6272 = 49*128). dst == repeat(arange(N), 16): 16 in-edges per node.
  - Per layer each core computes feat = x_shard @ W (fp32 PE) plus the
    attention dot products el/er. Packed fp16 PAIR-BLOCK table rows
    [feat(2j)|el(2j) || feat(2j+1)|el(2j+1) || pad] are AllGather'd so
    every core holds the full table.
  - Edge phase: per 128-node group (2048 edges), two 1024-index
    dma_gather instructions (int16 pair indices = src//2, spread over 4
    SWDGE queues) fetch one 528B pair block per edge; one predicated DVE
    copy selects the wanted half in place. Everything downstream is
    node-partition-aligned: leaky-relu logits, exp, per-node softmax
    denominators, weighted sums over the 16 edge slots, bias/relu.
  - er never leaves the core; el rides in the table (computed fp32 in
    the feat phase). Softmax skips max-subtraction (logits are O(1)).
"""

import os
import numpy as np

# ---- fixed problem dims -------------------------------------------------
N = 50000
DEG = 16
IN = 256
HID = 32
HEAD = 4
OUT = 40
HH = HID * HEAD  # 128
NEG_SLOPE = 0.2
NCORES = 8
NV = N // NCORES          # 6250 valid nodes per core

SUB12 = HH + 4            # 132 fp16 payload per node row (layers 1/2)
BLK12 = 384               # fp16 per pair block (768B stride)
SUB3 = OUT + 1            # 41
BLK3 = 128                # 256B stride
NIDX = 1024               # indices per dma_gather (2048 crashes the ring)

_PROGRAM_CACHE = {}
LAST_RESULTS = None


def _dma_gather_raw(nc, mybir, out_ap, in_ap, idxs_ap, num_idxs, elem_size,
                    elem_step, queue_num=0):
    """dma_gather minus the over-strict elem%256B assert (stride must still
    be a 256B multiple; verified on HW with 528B/164B elems)."""
    eng = nc.gpsimd
    stride_bytes = elem_step * mybir.dt.size(in_ap.dtype)
    assert stride_bytes % 256 == 0 and stride_bytes // 256 < 256
    _in_ap = eng.lower_ap_dma(in_ap, for_custom_bir_dma=True)
    _idxs_ap = eng.lower_ap(idxs_ap)
    _out_ap = eng.lower_ap(out_ap)
    return eng.add_instruction(
        mybir.InstDMAGatherAnt(
            name=nc.get_next_instruction_name(),
            ins=[*_in_ap, _idxs_ap,
                 eng.lower_val_access(eng.to_reg(num_idxs))],
            outs=[_out_ap],
            transpose=False, num_idxs=num_idxs, elem_size=elem_size,
            stride_bytes_256=stride_bytes // 256, gen_mode=0,
            single_packet=True, queue_num=queue_num,
            sbuf_tokens_per_rank=0, sbuf_free_dim_per_rank=0,
            sbuf_free_dim_pad_per_rank=0, sbuf_byte_offset=0,
        ))


# ========================================================================
# device program
# ========================================================================
def _build_program(ncores: int, ns_pad: int):
    from concourse import bass, mybir, tile, bacc
    from concourse.masks import make_identity
    from concourse.library_config import mlp

    f32 = mybir.dt.float32
    f16 = mybir.dt.float16
    i16 = mybir.dt.int16
    AX = mybir.AxisListType
    OPT = mybir.AluOpType
    AF = mybir.ActivationFunctionType

    G = ns_pad // 128
    NT = ncores * ns_pad          # table rows
    NB = NT // 2                  # pair blocks
    nsb = ns_pad // 2             # shard pair blocks

    nc = bacc.Bacc(
        "TRN2", target_bir_lowering=False, debug=False,
        enable_asserts=False, num_devices=ncores, num_swdge_queues=4)

    # ---- kernel I/O ----
    x0_d = nc.dram_tensor("x0", [ns_pad, IN], f32, kind="ExternalInput").ap()
    idx_d = nc.dram_tensor("idx", [128, G * 128], i16,
                           kind="ExternalInput").ap()
    sel_d = nc.dram_tensor("sel", [128, G * DEG], mybir.dt.uint8,
                           kind="ExternalInput").ap()
    w1_d = nc.dram_tensor("w1", [IN, HH + 2 * HEAD], f32,
                          kind="ExternalInput").ap()
    wh_d = nc.dram_tensor("wh", [HH, HH + 2], f32, kind="ExternalInput").ap()
    w2_d = nc.dram_tensor("w2", [HH, OUT + 2], f32,
                          kind="ExternalInput").ap()
    cst = {}
    for nm, w in [("al1", HH), ("ar1", HH), ("b1", HH),
                  ("alh", HH), ("arh", HH), ("bh", HH),
                  ("al2", OUT), ("ar2", OUT), ("b2", OUT)]:
        cst[nm] = nc.dram_tensor(nm, [128, w], f32, kind="ExternalInput").ap()
    out_d = nc.dram_tensor("out", [ns_pad, OUT], f32,
                           kind="ExternalOutput").ap()

    shared = "Shared" if ncores > 4 else "Local"
    gs1_d = nc.dram_tensor("gs1", [nsb, BLK12], f16).ap()
    gs2_d = nc.dram_tensor("gs2", [nsb, BLK12], f16).ap()
    gs3_d = nc.dram_tensor("gs3", [nsb, BLK3], f16).ap()
    gf1_d = nc.dram_tensor("gf1", [NB, BLK12], f16, addr_space=shared).ap()
    gf2_d = nc.dram_tensor("gf2", [NB, BLK12], f16, addr_space=shared).ap()
    gf3_d = nc.dram_tensor("gf3", [NB, BLK3], f16, addr_space=shared).ap()
    hs1_d = nc.dram_tensor("hs1", [ns_pad, HH], f16).ap()
    hs2_d = nc.dram_tensor("hs2", [ns_pad, HH], f16).ap()

    rgroups = [list(range(ncores))]

    with tile.TileContext(nc) as tc:
        with (
            tc.tile_pool(name="const", bufs=1) as cp,
            tc.tile_pool(name="feat", bufs=3) as fp,
            tc.tile_pool(name="edge", bufs=3) as ep,
            tc.tile_pool(name="psum", bufs=2, space="PSUM") as pp,
        ):
            nc.gpsimd.load_library(mlp)
            ident = cp.tile([128, 128], f32)
            make_identity(nc, ident[:])
            idx_sb = cp.tile([128, G * 128], i16)
            nc.sync.dma_start(out=idx_sb[:], in_=idx_d[:, :])
            sel_sb = cp.tile([128, G * DEG], mybir.dt.uint8)
            nc.sync.dma_start(out=sel_sb[:], in_=sel_d[:, :])
            w1a = cp.tile([128, HH + 2 * HEAD], f32)
            w1b = cp.tile([128, HH + 2 * HEAD], f32)
            nc.sync.dma_start(out=w1a[:], in_=w1_d[0:128, :])
            nc.sync.dma_start(out=w1b[:], in_=w1_d[128:256, :])
            wh_sb = cp.tile([128, HH + 2], f32)
            nc.sync.dma_start(out=wh_sb[:], in_=wh_d[:, :])
            w2_sb = cp.tile([128, OUT + 2], f32)
            nc.sync.dma_start(out=w2_sb[:], in_=w2_d[:, :])
            ct = {}
            for nm, w in [("al1", HH), ("ar1", HH), ("b1", HH),
                          ("alh", HH), ("arh", HH), ("bh", HH),
                          ("al2", OUT), ("ar2", OUT), ("b2", OUT)]:
                t = cp.tile([128, w], f32, name=f"c_{nm}")
                nc.sync.dma_start(out=t[:], in_=cst[nm][:, :])
                ct[nm] = t
            er1 = cp.tile([128, G * HEAD], f32)
            er2 = cp.tile([128, G], f32)
            er3 = cp.tile([128, G], f32)

            def gat_layer(lname, x_d, x16, K, SUB, BLK, HD, H,
                          w_tiles, al_t, ar_t, b_t, er_t, gs_d, gf_d, h_out):
                D = HD // H
                nchunk = K // 128
                # ---------------- feat phase ----------------
                for g in range(G):
                    r0, r1 = g * 128, (g + 1) * 128
                    x_t = fp.tile([128, K], f16 if x16 else f32,
                                  tag="x_t", name=f"{lname}_x{g}")
                    nc.sync.dma_start(out=x_t[:], in_=x_d[r0:r1, :])
                    if x16:
                        xf = fp.tile([128, K], f32, tag="xf",
                                     name=f"{lname}_xf{g}")
                        nc.vector.tensor_copy(xf[:], x_t[:])
                        x_t = xf
                    NW = HD + 2 * H
                    feat_ps = pp.tile([128, NW], f32, tag="feat_ps",
                                      name=f"{lname}_fps{g}")
                    for c in range(nchunk):
                        xT_ps = pp.tile([128, 128], f32, tag="xT_ps",
                                        name=f"{lname}_xtp{g}_{c}")
                        nc.tensor.transpose(
                            xT_ps[:], x_t[:, c * 128:(c + 1) * 128], ident[:])
                        xT_sb = fp.tile([128, 128], f32, tag="xT_sb",
                                        name=f"{lname}_xts{g}_{c}")
                        nc.vector.tensor_copy(xT_sb[:], xT_ps[:])
                        nc.tensor.matmul(
                            feat_ps[:], lhsT=xT_sb[:], rhs=w_tiles[c][:],
                            start=(c == 0), stop=(c == nchunk - 1))
                    grow = fp.tile([128, SUB], f16, tag="grow",
                                   name=f"{lname}_grow{g}")
                    nc.vector.tensor_copy(grow[:, 0:HD + H],
                                          feat_ps[:, 0:HD + H])
                    nc.vector.tensor_copy(
                        er_t[:, g * H:(g + 1) * H],
                        feat_ps[:, HD + H:HD + 2 * H])
                    # 128 node rows -> 64 pair blocks; sub-rows at 0 and SUB
                    dst = gs_d[g * 64:(g + 1) * 64, 0:2 * SUB].rearrange(
                        "b (s c) -> b s c", c=SUB)
                    nc.sync.dma_start(out=dst, in_=grow[:])

                # ---------------- all-gather ----------------
                nc.gpsimd.collective_compute(
                    "AllGather", OPT.bypass, replica_groups=rgroups,
                    ins=[gs_d[:, :]], outs=[gf_d[:, :]])

                # ---------------- edge phase ----------------
                mode, o_d = h_out
                ELEM = 2 * SUB
                for g in range(G):
                    r0, r1 = g * 128, (g + 1) * 128
                    big = ep.tile([128, DEG * ELEM], f16, tag="big",
                                  bufs=4, name=f"{lname}_big{g}")
                    for h in range(2):
                        _dma_gather_raw(
                            nc, mybir,
                            big[:, h * 8 * ELEM:(h + 1) * 8 * ELEM],
                            gf_d[:, 0:ELEM],
                            idx_sb[:, g * 128 + h * 64:g * 128 + (h + 1) * 64],
                            NIDX, ELEM, BLK, queue_num=(2 * g + h) % 4)
                    bv = big[:].rearrange("p (k r) -> p k r", r=ELEM)
                    lo = bv[:, :, 0:SUB]
                    hi = bv[:, :, SUB:2 * SUB]
                    mask = (sel_sb[:, g * DEG:(g + 1) * DEG]
                            .unsqueeze(2).to_broadcast((128, DEG, SUB)))
                    nc.vector.copy_predicated(out=lo, mask=mask, data=hi)
                    feat_e = bv[:, :, 0:HD]          # [128, 16, HD] fp16
                    el_e = bv[:, :, HD:HD + H]       # [128, 16, H] fp16
                    e_t = ep.tile([128, DEG * H], f32, tag="e_t",
                                  name=f"{lname}_et{g}")
                    etv = e_t[:].rearrange("p (k h) -> p k h", h=H)
                    if H == 1:
                        nc.scalar.activation(
                            out=etv, in_=el_e, func=AF.Identity,
                            bias=er_t[:, g:g + 1])
                    else:
                        erv = (er_t[:, g * H:(g + 1) * H]
                               .unsqueeze(1).to_broadcast((128, DEG, H)))
                        nc.vector.tensor_tensor(
                            out=etv, in0=el_e, in1=erv, op=OPT.add)
                    e2 = ep.tile([128, DEG * H], f32, tag="e2",
                                 name=f"{lname}_e2{g}")
                    nc.vector.scalar_tensor_tensor(
                        out=e2[:], in0=e_t[:], scalar=NEG_SLOPE, in1=e_t[:],
                        op0=OPT.mult, op1=OPT.max)
                    ex = ep.tile([128, DEG * H], f32, tag="ex",
                                 name=f"{lname}_ex{g}")
                    den = ep.tile([128, H], f32, tag="den",
                                  name=f"{lname}_den{g}")
                    if H == 1:
                        nc.scalar.activation(out=ex[:], in_=e2[:],
                                             func=AF.Exp, accum_out=den[:])
                    else:
                        nc.scalar.activation(out=ex[:], in_=e2[:], func=AF.Exp)
                        dt_ = ep.tile([128, 8 * H], f32, tag="dt",
                                      name=f"{lname}_dt{g}")
                        nc.vector.tensor_tensor(
                            out=dt_[:], in0=ex[:, 0:8 * H],
                            in1=ex[:, 8 * H:16 * H], op=OPT.add)
                        for wdt in (4 * H, 2 * H, H):
                            nc.vector.tensor_tensor(
                                out=dt_[:, 0:wdt], in0=dt_[:, 0:wdt],
                                in1=dt_[:, wdt:2 * wdt], op=OPT.add)
                        nc.vector.tensor_copy(den[:], dt_[:, 0:H])
                    inv = ep.tile([128, H], f32, tag="inv",
                                  name=f"{lname}_inv{g}")
                    nc.vector.reciprocal(inv[:], den[:])
                    f_all = ep.tile([128, DEG * HD], f16, tag="f_all",
                                    name=f"{lname}_fa{g}")
                    featv = feat_e.rearrange("p k (h d) -> p k h d", h=H)
                    exv = (ex[:].rearrange("p (k h) -> p k h", h=H)
                           .unsqueeze(3).to_broadcast((128, DEG, H, D)))
                    nc.vector.tensor_tensor(
                        out=f_all[:].rearrange("p (k h d) -> p k h d",
                                               k=DEG, h=H),
                        in0=featv, in1=exv, op=OPT.mult)
                    u32 = ep.tile([128, 8 * HD], f32, tag="u",
                                  name=f"{lname}_u{g}")
                    nc.vector.tensor_tensor(
                        out=u32[:], in0=f_all[:, 0:8 * HD],
                        in1=f_all[:, 8 * HD:16 * HD], op=OPT.add)
                    for wdt in (4 * HD, 2 * HD, HD):
                        nc.vector.tensor_tensor(
                            out=u32[:, 0:wdt], in0=u32[:, 0:wdt],
                            in1=u32[:, wdt:2 * wdt], op=OPT.add)
                    u = u32[:, 0:HD]
                    if mode == "relu":
                        ht = ep.tile([128, HD], f32, tag="ht",
                                     name=f"{lname}_ht{g}")
                        if H == 1:
                            nc.vector.scalar_tensor_tensor(
                                out=ht[:], in0=u, scalar=inv[:, 0:1],
                                in1=b_t[:, 0:HD], op0=OPT.mult, op1=OPT.add)
                        else:
                            invv = inv[:].unsqueeze(2).to_broadcast(
                                (128, H, D))
                            t1 = ep.tile([128, HD], f32, tag="t1",
                                         name=f"{lname}_t1{g}")
                            nc.vector.tensor_tensor(
                                out=t1[:].rearrange("p (h d) -> p h d", h=H),
                                in0=u.rearrange("p (h d) -> p h d", h=H),
                                in1=invv, op=OPT.mult)
                            nc.vector.tensor_tensor(
                                out=ht[:], in0=t1[:], in1=b_t[:, 0:HD],
                                op=OPT.add)
                        hrelu = ep.tile([128, HD], f16, tag="hrelu",
                                        name=f"{lname}_hr{g}")
                        nc.scalar.activation(out=hrelu[:], in_=ht[:],
                                             func=AF.Relu)
                        nc.sync.dma_start(out=o_d[r0:r1, :], in_=hrelu[:])
                    else:  # logsoftmax (final layer)
                        ht = ep.tile([128, HD], f32, tag="ht",
                                     name=f"{lname}_ht{g}")
                        nc.vector.scalar_tensor_tensor(
                            out=ht[:], in0=u, scalar=inv[:, 0:1],
                            in1=b_t[:, 0:HD], op0=OPT.mult, op1=OPT.add)
                        nm_t = ep.tile([128, 1], f32, tag="nm",
                                       name=f"{lname}_nm{g}")
                        nc.vector.reduce_max(out=nm_t[:], in_=ht[:],
                                             axis=AX.X, negate=True)
                        exf = ep.tile([128, HD], f32, tag="exf",
                                      name=f"{lname}_exf{g}")
                        s_t = ep.tile([128, 1], f32, tag="s_t",
                                      name=f"{lname}_s{g}")
                        nc.scalar.activation(out=exf[:], in_=ht[:],
                                             func=AF.Exp, bias=nm_t[:],
                                             accum_out=s_t[:])
                        ls = ep.tile([128, 1], f32, tag="ls",
                                     name=f"{lname}_ls{g}")
                        nc.scalar.activation(out=ls[:], in_=s_t[:],
                                             func=AF.Ln)
                        o_t = ep.tile([128, HD], f32, tag="o_t",
                                      name=f"{lname}_o{g}")
                        nc.vector.scalar_tensor_tensor(
                            out=o_t[:], in0=ht[:], scalar=nm_t[:],
                            in1=ls[:].to_broadcast((128, HD)),
                            op0=OPT.add, op1=OPT.subtract)
                        nc.sync.dma_start(out=o_d[r0:r1, :], in_=o_t[:])

            gat_layer("L1", x0_d, False, IN, SUB12, BLK12, HH, HEAD,
                      [w1a, w1b], ct["al1"], ct["ar1"], ct["b1"], er1,
                      gs1_d, gf1_d, ("relu", hs1_d))
            gat_layer("L2", hs1_d, True, HH, SUB12, BLK12, HH, 1,
                      [wh_sb], ct["alh"], ct["arh"], ct["bh"], er2,
                      gs2_d, gf2_d, ("relu", hs2_d))
            gat_layer("L3", hs2_d, True, HH, SUB3, BLK3, OUT, 1,
                      [w2_sb], ct["al2"], ct["ar2"], ct["b2"], er3,
                      gs3_d, gf3_d, ("logsoftmax", out_d))

    nc.compile()
    return nc


# ========================================================================
# host side
# ========================================================================
def _get_program(ncores, ns_pad):
    key = (ncores, ns_pad)
    if key not in _PROGRAM_CACHE:
        _PROGRAM_CACHE[key] = _build_program(ncores, ns_pad)
    return _PROGRAM_CACHE[key]


def _numpy_fallback(feats, src, dst, W1, al1, ar1, b1, Wh, alh, arh, bh,
                    W2, al2, ar2, b2):
    n = feats.shape[0]

    def gat(x, W, al, ar, b):
        Hh, Dd = al.shape
        feat = (x @ W).reshape(n, Hh, Dd)
        el = (feat * al).sum(-1)
        er = (feat * ar).sum(-1)
        e = el[src] + er[dst]
        e = np.where(e > 0, e, NEG_SLOPE * e).astype(np.float32)
        emax = np.full((n, Hh), -np.inf, np.float32)
        np.maximum.at(emax, dst, e)
        ex = np.exp(e - emax[dst])
        den = np.zeros((n, Hh), np.float32)
        np.add.at(den, dst, ex)
        alpha = ex / den[dst]
        out = np.zeros((n, Hh, Dd), np.float32)
        np.add.at(out, dst, feat[src] * alpha[..., None])
        return out + b.reshape(1, Hh, Dd)

    h = np.maximum(gat(feats, W1, al1, ar1, b1).reshape(n, HH), 0.0)
    h = np.maximum(gat(h, Wh, alh, arh, bh).mean(1), 0.0)
    h = gat(h, W2, al2, ar2, b2).mean(1)
    m = h.max(1, keepdims=True)
    ls = np.log(np.exp(h - m).sum(1, keepdims=True))
    return (h - m - ls).astype(np.float32)


def _prep_core_inputs(feats, src_tbl, r, nv, ns_pad, common):
    G = ns_pad // 128
    x = np.zeros((ns_pad, IN), np.float32)
    x[:nv] = feats[r * nv:(r + 1) * nv]
    sp = np.zeros(ns_pad * DEG, np.int64)
    sp[:nv * DEG] = src_tbl[r * nv * DEG:(r + 1) * nv * DEG]
    # edge (node m, slot k): m = g*128 + p.  Gather h covers slots 8h..8h+8;
    # its list position i maps to (p = i%128, j = i//128, k = 8h+j).
    e = sp.reshape(G, 128, DEG)                  # [g, p, k]
    idx = np.zeros((128, G * 128), np.int16)
    sel = np.zeros((128, G * DEG), np.uint8)
    for g in range(G):
        for h in range(2):
            lst = (e[g, :, 8 * h:8 * h + 8] // 2).astype(
                np.int16).T.reshape(-1)          # i = j*128 + p
            a = lst.reshape(64, 16).T            # [16, 64]
            idx[:, g * 128 + h * 64:g * 128 + (h + 1) * 64] = np.tile(
                a, (8, 1))
        sel[:, g * DEG:(g + 1) * DEG] = (e[g] % 2).astype(np.uint8)
    return dict(x0=x, idx=idx, sel=sel, **common)


def kernel(**inputs) -> np.ndarray:
    global LAST_RESULTS
    feats = np.ascontiguousarray(np.asarray(inputs["features"],
                                            dtype=np.float32))
    src = np.asarray(inputs["src"]).astype(np.int64).ravel()
    dst = np.asarray(inputs["dst"]).astype(np.int64).ravel()
    W1 = np.asarray(inputs["W1"], dtype=np.float32)
    al1 = np.asarray(inputs["al1"], dtype=np.float32)
    ar1 = np.asarray(inputs["ar1"], dtype=np.float32)
    b1 = np.asarray(inputs["b1"], dtype=np.float32)
    Wh = np.asarray(inputs["Wh"], dtype=np.float32)
    alh = np.asarray(inputs["alh"], dtype=np.float32)
    arh = np.asarray(inputs["arh"], dtype=np.float32)
    bh = np.asarray(inputs["bh"], dtype=np.float32)
    W2 = np.asarray(inputs["W2"], dtype=np.float32)
    al2 = np.asarray(inputs["al2"], dtype=np.float32)
    ar2 = np.asarray(inputs["ar2"], dtype=np.float32)
    b2 = np.asarray(inputs["b2"], dtype=np.float32)

    n = feats.shape[0]
    expected_dst = np.repeat(np.arange(N, dtype=np.int64), DEG)
    if (n != N or src.shape[0] != N * DEG
            or not np.array_equal(dst, expected_dst)
            or src.min() < 0 or src.max() >= N):
        return _numpy_fallback(feats, src, dst, W1, al1, ar1, b1,
                               Wh, alh, arh, bh, W2, al2, ar2, b2)

    from concourse.bass_utils import run_bass_kernel_spmd

    G = (NV + 127) // 128
    ns_pad = G * 128  # 6272
    nc = _get_program(NCORES, ns_pad)

    src_tbl = (src // NV) * ns_pad + (src % NV)   # table row ids

    def bcast(a, w):
        return np.ascontiguousarray(
            np.broadcast_to(a.reshape(1, w), (128, w)).astype(np.float32))

    def ext(W, al, ar):
        Hh, Dd = al.shape
        Wr = W.reshape(W.shape[0], Hh, Dd)
        wal = np.einsum("khd,hd->kh", Wr, al).astype(np.float32)
        war = np.einsum("khd,hd->kh", Wr, ar).astype(np.float32)
        return np.ascontiguousarray(
            np.concatenate([W, wal, war], axis=1).astype(np.float32))

    common = dict(
        w1=ext(W1, al1, ar1), wh=ext(Wh, alh, arh), w2=ext(W2, al2, ar2),
        al1=bcast(al1, HH), ar1=bcast(ar1, HH), b1=bcast(b1, HH),
        alh=bcast(alh, HH), arh=bcast(arh, HH), bh=bcast(bh, HH),
        al2=bcast(al2, OUT), ar2=bcast(ar2, OUT), b2=bcast(b2, OUT),
    )
    in_maps = [
        _prep_core_inputs(feats, src_tbl, r, NV, ns_pad, common)
        for r in range(NCORES)
    ]

    trace = bool(int(os.environ.get("GAT_TRACE", "0")))
    LAST_RESULTS = run_bass_kernel_spmd(
        nc, in_maps, list(range(NCORES)), trace=trace)
    outs = [LAST_RESULTS.results[r]["out"][:NV] for r in range(NCORES)]
    return np.ascontiguousarray(np.concatenate(outs, axis=0),
                                dtype=np.float32)



# revision 3
# speedup vs baseline: 1.0717x; 1.0717x over previous
"""Trainium2 Bass kernel: 3-layer GAT (nn_GAT_62182536511748).

Strategy (8 NeuronCores, SPMD, fp16 pair-block gather), v2:
  - Nodes sharded contiguously across cores (6250 valid/core, padded to
    6272 = 49*128). dst == repeat(arange(N), 16): 16 in-edges per node.
  - Per layer each core computes feat = x_shard @ W (fp16 PE) plus the
    attention dot products el/er. The gather-table rows hold feat+bias
    (bias folded in: sum(alpha)=1 makes this exact) and el. Rows are
    packed in fp16 PAIR blocks and AllGather'd in 4 CHUNKS, each fired
    as soon as its feat groups are done so collectives overlap compute.
  - Edge phase per 128-node group (2048 edges): two 1024-index
    dma_gather instructions fetch one pair block per edge; one
    predicated DVE copy over an int32 bitcast view (half the walk of
    fp16) selects the wanted half. Logits/exp on scalar engine, the
    weighted sum is an fp16 multiply + fp16 pairwise tree, and the
    1/den scale rides the scalar-engine relu via per-partition scale.
  - L1 feat consumes a host-pretransposed fp16 x0T (no PE transposes).
    L2/L3 feat transpose h via PE with an fp16 identity; PSUM->SBUF
    copies run on the scalar engine to keep DVE clear.
  - L3 log_softmax: per-group exp keeps the scalar engine on one
    activation table; ln(sum) is batched over all 49 groups at the end
    (one table switch total) and the output leaves in one big DMA.
"""

import os
import numpy as np

# ---- fixed problem dims -------------------------------------------------
N = 50000
DEG = 16
IN = 256
HID = 32
HEAD = 4
OUT = 40
HH = HID * HEAD  # 128
NEG_SLOPE = 0.2
NCORES = 8
NV = N // NCORES          # 6250 valid nodes per core

SUB12 = HH + 4            # 132 fp16 payload per node row (layers 1/2)
BLK12 = 384               # fp16 per pair block (768B stride)
SUB3 = OUT + 2            # 42 (feat 40 | el 1 | pad 1) -> 84B, 4B aligned
BLK3 = 128                # 256B stride
NIDX = 1024               # indices per dma_gather (2048 crashes the ring)
CHG = [13, 12, 12, 12]    # feat groups per all-gather chunk (sum = 49)

_PROGRAM_CACHE = {}
LAST_RESULTS = None


def _dma_gather_raw(nc, mybir, out_ap, in_ap, idxs_ap, num_idxs, elem_size,
                    elem_step, queue_num=0):
    """dma_gather minus the over-strict elem%256B assert (stride must still
    be a 256B multiple; verified on HW with 528B/164B elems)."""
    eng = nc.gpsimd
    stride_bytes = elem_step * mybir.dt.size(in_ap.dtype)
    assert stride_bytes % 256 == 0 and stride_bytes // 256 < 256
    _in_ap = eng.lower_ap_dma(in_ap, for_custom_bir_dma=True)
    _idxs_ap = eng.lower_ap(idxs_ap)
    _out_ap = eng.lower_ap(out_ap)
    return eng.add_instruction(
        mybir.InstDMAGatherAnt(
            name=nc.get_next_instruction_name(),
            ins=[*_in_ap, _idxs_ap,
                 eng.lower_val_access(eng.to_reg(num_idxs))],
            outs=[_out_ap],
            transpose=False, num_idxs=num_idxs, elem_size=elem_size,
            stride_bytes_256=stride_bytes // 256, gen_mode=0,
            single_packet=True, queue_num=queue_num,
            sbuf_tokens_per_rank=0, sbuf_free_dim_per_rank=0,
            sbuf_free_dim_pad_per_rank=0, sbuf_byte_offset=0,
        ))


# ========================================================================
# device program
# ========================================================================
def _build_program(ncores: int, ns_pad: int):
    from concourse import bass, mybir, tile, bacc
    from concourse.masks import make_identity
    from concourse.library_config import mlp

    f32 = mybir.dt.float32
    f16 = mybir.dt.float16
    i16 = mybir.dt.int16
    i32 = mybir.dt.int32
    AX = mybir.AxisListType
    OPT = mybir.AluOpType
    AF = mybir.ActivationFunctionType

    G = ns_pad // 128
    NT = ncores * ns_pad          # table rows
    NB = NT // 2                  # pair blocks
    nsb = ns_pad // 2             # shard pair blocks
    NCH = len(CHG)
    GS0 = [sum(CHG[:c]) for c in range(NCH)]          # first group of chunk
    B0 = [g * 64 for g in GS0]                        # first local block
    NBLK = [c * 64 for c in CHG]                      # blocks per chunk
    GOFF = [8 * b for b in B0]                        # global block offset

    nc = bacc.Bacc(
        "TRN2", target_bir_lowering=False, debug=False,
        enable_asserts=False, num_devices=ncores, num_swdge_queues=4)

    # ---- kernel I/O ----
    x0t_d = nc.dram_tensor("x0t", [IN, ns_pad], f16, kind="ExternalInput").ap()
    idx_d = nc.dram_tensor("idx", [128, G * 128], i16,
                           kind="ExternalInput").ap()
    sel_d = nc.dram_tensor("sel", [128, G * DEG], mybir.dt.uint8,
                           kind="ExternalInput").ap()
    w1_d = nc.dram_tensor("w1", [IN, HH + 2 * HEAD], f16,
                          kind="ExternalInput").ap()
    wh_d = nc.dram_tensor("wh", [HH, HH + 2], f16, kind="ExternalInput").ap()
    w2_d = nc.dram_tensor("w2", [HH, OUT + 2], f16,
                          kind="ExternalInput").ap()
    cst = {}
    for nm, w in [("b1", HH), ("bh", HH), ("b2", OUT)]:
        cst[nm] = nc.dram_tensor(nm, [128, w], f32, kind="ExternalInput").ap()
    out_d = nc.dram_tensor("out", [ns_pad, OUT], f32,
                           kind="ExternalOutput").ap()

    shared = "Shared" if ncores > 4 else "Local"
    gs_d = {}   # (layer, chunk) -> shard chunk tensor
    for L, blk in ((1, BLK12), (2, BLK12), (3, BLK3)):
        for c in range(NCH):
            gs_d[(L, c)] = nc.dram_tensor(
                f"gs{L}_{c}", [NBLK[c], blk], f16).ap()
    gf1_d = nc.dram_tensor("gf1", [NB, BLK12], f16, addr_space=shared).ap()
    gf2_d = nc.dram_tensor("gf2", [NB, BLK12], f16, addr_space=shared).ap()
    gf3_d = nc.dram_tensor("gf3", [NB, BLK3], f16, addr_space=shared).ap()
    gf = {1: gf1_d, 2: gf2_d, 3: gf3_d}
    hs1_d = nc.dram_tensor("hs1", [ns_pad, HH], f16).ap()
    hs2_d = nc.dram_tensor("hs2", [ns_pad, HH], f16).ap()

    rgroups = [list(range(ncores))]

    with tile.TileContext(nc) as tc:
        with (
            tc.tile_pool(name="const", bufs=1) as cp,
            tc.tile_pool(name="feat", bufs=3) as fp,
            tc.tile_pool(name="edge", bufs=3) as ep,
            tc.tile_pool(name="psum", bufs=2, space="PSUM") as pp,
        ):
            nc.gpsimd.load_library(mlp)
            ident = cp.tile([128, 128], f16)
            make_identity(nc, ident[:])
            idx_sb = cp.tile([128, G * 128], i16)
            nc.sync.dma_start(out=idx_sb[:], in_=idx_d[:, :])
            sel_sb = cp.tile([128, G * DEG], mybir.dt.uint8)
            nc.sync.dma_start(out=sel_sb[:], in_=sel_d[:, :])
            w1a = cp.tile([128, HH + 2 * HEAD], f16)
            w1b = cp.tile([128, HH + 2 * HEAD], f16)
            nc.sync.dma_start(out=w1a[:], in_=w1_d[0:128, :])
            nc.sync.dma_start(out=w1b[:], in_=w1_d[128:256, :])
            wh_sb = cp.tile([128, HH + 2], f16)
            nc.sync.dma_start(out=wh_sb[:], in_=wh_d[:, :])
            w2_sb = cp.tile([128, OUT + 2], f16)
            nc.sync.dma_start(out=w2_sb[:], in_=w2_d[:, :])
            ct = {}
            for nm, w in [("b1", HH), ("bh", HH), ("b2", OUT)]:
                t = cp.tile([128, w], f32, name=f"c_{nm}")
                nc.sync.dma_start(out=t[:], in_=cst[nm][:, :])
                ct[nm] = t
            er1 = cp.tile([128, G * HEAD], f32)
            er2 = cp.tile([128, G], f32)
            er3 = cp.tile([128, G], f32)
            # L3 log-softmax batching state
            ht_all = cp.tile([128, G * OUT], f32)
            o_all = cp.tile([128, G * OUT], f32)
            nm_all = cp.tile([128, G], f32)
            s_all = cp.tile([128, G], f32)
            ls_all = cp.tile([128, G], f32)

            LCFG = {
                1: dict(K=IN, SUB=SUB12, BLK=BLK12, HD=HH, H=HEAD,
                        w=[w1a, w1b], b=ct["b1"], er=er1, x=None),
                2: dict(K=HH, SUB=SUB12, BLK=BLK12, HD=HH, H=1,
                        w=[wh_sb], b=ct["bh"], er=er2, x=hs1_d),
                3: dict(K=HH, SUB=SUB3, BLK=BLK3, HD=OUT, H=1,
                        w=[w2_sb], b=ct["b2"], er=er3, x=hs2_d),
            }

            def feat_group(L, g):
                cfg = LCFG[L]
                K, SUB, HD, H = cfg["K"], cfg["SUB"], cfg["HD"], cfg["H"]
                r0, r1 = g * 128, (g + 1) * 128
                NW = HD + 2 * H
                feat_ps = pp.tile([128, NW], f32, tag=f"fps{L}",
                                  name=f"L{L}_fps{g}")
                if L == 1:
                    for c in range(2):
                        lt = fp.tile([128, 128], f16, tag="lhsT",
                                     name=f"L1_lt{g}_{c}")
                        nc.sync.dma_start(
                            out=lt[:],
                            in_=x0t_d[c * 128:(c + 1) * 128, r0:r1])
                        nc.tensor.matmul(
                            feat_ps[:], lhsT=lt[:], rhs=cfg["w"][c][:],
                            start=(c == 0), stop=(c == 1))
                else:
                    x_t = fp.tile([128, K], f16, tag="x_t",
                                  name=f"L{L}_x{g}")
                    nc.sync.dma_start(out=x_t[:], in_=cfg["x"][r0:r1, :])
                    xT_ps = pp.tile([128, 128], f16, tag="xT_ps",
                                    name=f"L{L}_xtp{g}")
                    nc.tensor.transpose(xT_ps[:], x_t[:], ident[:])
                    xT_sb = fp.tile([128, 128], f16, tag="xT_sb",
                                    name=f"L{L}_xts{g}")
                    nc.scalar.copy(xT_sb[:], xT_ps[:])
                    nc.tensor.matmul(
                        feat_ps[:], lhsT=xT_sb[:], rhs=cfg["w"][0][:],
                        start=True, stop=True)
                grow = fp.tile([128, SUB], f16, tag=f"grow{L}",
                               name=f"L{L}_grow{g}")
                # bias folded into the table: sum(alpha)=1 keeps it exact
                nc.vector.tensor_tensor(
                    out=grow[:, 0:HD], in0=feat_ps[:, 0:HD],
                    in1=cfg["b"][:, 0:HD], op=OPT.add)
                nc.scalar.copy(grow[:, HD:HD + H], feat_ps[:, HD:HD + H])
                nc.scalar.copy(cfg["er"][:, g * H:(g + 1) * H],
                               feat_ps[:, HD + 2 * H - H:HD + 2 * H])
                c = next(i for i in range(NCH)
                         if GS0[i] <= g < GS0[i] + CHG[i])
                b_lo = (g - GS0[c]) * 64
                dst = gs_d[(L, c)][b_lo:b_lo + 64, 0:2 * SUB].rearrange(
                    "b (s c) -> b s c", c=SUB)
                nc.sync.dma_start(out=dst, in_=grow[:])

            def allgather(L, c):
                cfg = LCFG[L]
                nc.gpsimd.collective_compute(
                    "AllGather", OPT.bypass, replica_groups=rgroups,
                    ins=[gs_d[(L, c)][:, :]],
                    outs=[gf[L][GOFF[c]:GOFF[c] + 8 * NBLK[c], :]])

            def edge_group(L, g):
                cfg = LCFG[L]
                SUB, BLK, HD, H = cfg["SUB"], cfg["BLK"], cfg["HD"], cfg["H"]
                er_t, b_t = cfg["er"], cfg["b"]
                D = HD // H
                r0, r1 = g * 128, (g + 1) * 128
                ELEM = 2 * SUB
                big = ep.tile([128, DEG * ELEM], f16, tag=f"big{L}",
                              bufs=4, name=f"L{L}_big{g}")
                for h in range(2):
                    _dma_gather_raw(
                        nc, mybir,
                        big[:, h * 8 * ELEM:(h + 1) * 8 * ELEM],
                        gf[L][:, 0:ELEM],
                        idx_sb[:, g * 128 + h * 64:g * 128 + (h + 1) * 64],
                        NIDX, ELEM, BLK, queue_num=(2 * g + h) % 4)
                # predicated half-select over an int32 view (half the walk)
                SUBI = SUB // 2
                bv32 = big[:].bitcast(i32).rearrange(
                    "p (k r) -> p k r", r=SUB)
                mask = (sel_sb[:, g * DEG:(g + 1) * DEG]
                        .unsqueeze(2).to_broadcast((128, DEG, SUBI)))
                nc.vector.copy_predicated(
                    out=bv32[:, :, 0:SUBI], mask=mask,
                    data=bv32[:, :, SUBI:SUB])
                bv = big[:].rearrange("p (k r) -> p k r", r=ELEM)
                feat_e = bv[:, :, 0:HD]          # [128, 16, HD] fp16 (+bias)
                el_e = bv[:, :, HD:HD + H]       # [128, 16, H] fp16
                e_t = ep.tile([128, DEG * H], f32, tag="e_t",
                              name=f"L{L}_et{g}")
                etv = e_t[:].rearrange("p (k h) -> p k h", h=H)
                if H == 1:
                    nc.scalar.activation(
                        out=etv, in_=el_e, func=AF.Identity,
                        bias=er_t[:, g:g + 1])
                else:
                    erv = (er_t[:, g * H:(g + 1) * H]
                           .unsqueeze(1).to_broadcast((128, DEG, H)))
                    nc.vector.tensor_tensor(
                        out=etv, in0=el_e, in1=erv, op=OPT.add)
                e2 = ep.tile([128, DEG * H], f32, tag="e2",
                             name=f"L{L}_e2{g}")
                nc.vector.scalar_tensor_tensor(
                    out=e2[:], in0=e_t[:], scalar=NEG_SLOPE, in1=e_t[:],
                    op0=OPT.mult, op1=OPT.max)
                ex = ep.tile([128, DEG * H], f32, tag="ex",
                             name=f"L{L}_ex{g}")
                den = ep.tile([128, H], f32, tag="den",
                              name=f"L{L}_den{g}")
                if H == 1:
                    nc.scalar.activation(out=ex[:], in_=e2[:],
                                         func=AF.Exp, accum_out=den[:])
                else:
                    nc.scalar.activation(out=ex[:], in_=e2[:], func=AF.Exp)
                    dt_ = ep.tile([128, 8 * H], f32, tag="dt",
                                  name=f"L{L}_dt{g}")
                    nc.vector.tensor_tensor(
                        out=dt_[:], in0=ex[:, 0:8 * H],
                        in1=ex[:, 8 * H:16 * H], op=OPT.add)
                    for wdt in (4 * H, 2 * H, H):
                        nc.vector.tensor_tensor(
                            out=dt_[:, 0:wdt], in0=dt_[:, 0:wdt],
                            in1=dt_[:, wdt:2 * wdt], op=OPT.add)
                    nc.vector.tensor_copy(den[:], dt_[:, 0:H])
                inv = ep.tile([128, H], f32, tag="inv",
                              name=f"L{L}_inv{g}")
                nc.vector.reciprocal(inv[:], den[:])
                # weighted (unnormalized) sum: fp16 multiply + fp16 tree
                f_all = ep.tile([128, DEG * HD], f16, tag=f"f_all{L}",
                                name=f"L{L}_fa{g}")
                featv = feat_e.rearrange("p k (h d) -> p k h d", h=H)
                exv = (ex[:].rearrange("p (k h) -> p k h", h=H)
                       .unsqueeze(3).to_broadcast((128, DEG, H, D)))
                nc.vector.tensor_tensor(
                    out=f_all[:].rearrange("p (k h d) -> p k h d",
                                           k=DEG, h=H),
                    in0=featv, in1=exv, op=OPT.mult)
                u16 = ep.tile([128, 8 * HD], f16, tag=f"u{L}",
                              name=f"L{L}_u{g}")
                nc.vector.tensor_tensor(
                    out=u16[:], in0=f_all[:, 0:8 * HD],
                    in1=f_all[:, 8 * HD:16 * HD], op=OPT.add)
                for wdt in (4 * HD, 2 * HD, HD):
                    nc.vector.tensor_tensor(
                        out=u16[:, 0:wdt], in0=u16[:, 0:wdt],
                        in1=u16[:, wdt:2 * wdt], op=OPT.add)
                u = u16[:, 0:HD]
                if L == 1:
                    t1 = ep.tile([128, HD], f16, tag="t1",
                                 name=f"L1_t1{g}")
                    invv = inv[:].unsqueeze(2).to_broadcast((128, H, D))
                    nc.vector.tensor_tensor(
                        out=t1[:].rearrange("p (h d) -> p h d", h=H),
                        in0=u.rearrange("p (h d) -> p h d", h=H),
                        in1=invv, op=OPT.mult)
                    hrelu = ep.tile([128, HD], f16, tag="hr1",
                                    name=f"L1_hr{g}")
                    nc.scalar.activation(out=hrelu[:], in_=t1[:],
                                         func=AF.Relu)
                    nc.sync.dma_start(out=hs1_d[r0:r1, :], in_=hrelu[:])
                elif L == 2:
                    hrelu = ep.tile([128, HD], f16, tag="hr2",
                                    name=f"L2_hr{g}")
                    nc.scalar.activation(out=hrelu[:], in_=u,
                                         func=AF.Relu, scale=inv[:, 0:1])
                    nc.sync.dma_start(out=hs2_d[r0:r1, :], in_=hrelu[:])
                else:
                    hts = ht_all[:, g * OUT:(g + 1) * OUT]
                    nc.scalar.activation(out=hts, in_=u,
                                         func=AF.Identity,
                                         scale=inv[:, 0:1])
                    nc.vector.reduce_max(out=nm_all[:, g:g + 1], in_=hts,
                                         axis=AX.X, negate=True)
                    exf = ep.tile([128, OUT], f32, tag="exf",
                                  name=f"L3_exf{g}")
                    nc.scalar.activation(out=exf[:], in_=hts,
                                         func=AF.Exp,
                                         bias=nm_all[:, g:g + 1],
                                         accum_out=s_all[:, g:g + 1])

            # ---------------- program ----------------
            # L1 feat, all-gather chunks fired as soon as ready
            for c in range(NCH):
                for g in range(GS0[c], GS0[c] + CHG[c]):
                    feat_group(1, g)
                allgather(1, c)
            # L1 edge + L2 feat fused; AG2 chunks fire mid-edge-phase
            for c in range(NCH):
                for g in range(GS0[c], GS0[c] + CHG[c]):
                    edge_group(1, g)
                    feat_group(2, g)
                allgather(2, c)
            for c in range(NCH):
                for g in range(GS0[c], GS0[c] + CHG[c]):
                    edge_group(2, g)
                    feat_group(3, g)
                allgather(3, c)
            for g in range(G):
                edge_group(3, g)
            # batched log-softmax tail: one Ln table load total
            nc.scalar.activation(out=ls_all[:], in_=s_all[:], func=AF.Ln)
            for g in range(G):
                nc.vector.scalar_tensor_tensor(
                    out=o_all[:, g * OUT:(g + 1) * OUT],
                    in0=ht_all[:, g * OUT:(g + 1) * OUT],
                    scalar=nm_all[:, g:g + 1],
                    in1=ls_all[:, g:g + 1].to_broadcast((128, OUT)),
                    op0=OPT.add, op1=OPT.subtract)
            nc.sync.dma_start(
                out=out_d[:, :].rearrange("(g p) d -> p g d", p=128),
                in_=o_all[:].rearrange("p (g d) -> p g d", d=OUT))

    nc.compile()
    return nc


# ========================================================================
# host side
# ========================================================================
def _get_program(ncores, ns_pad):
    key = (ncores, ns_pad)
    if key not in _PROGRAM_CACHE:
        _PROGRAM_CACHE[key] = _build_program(ncores, ns_pad)
    return _PROGRAM_CACHE[key]


def _numpy_fallback(feats, src, dst, W1, al1, ar1, b1, Wh, alh, arh, bh,
                    W2, al2, ar2, b2):
    n = feats.shape[0]

    def gat(x, W, al, ar, b):
        Hh, Dd = al.shape
        feat = (x @ W).reshape(n, Hh, Dd)
        el = (feat * al).sum(-1)
        er = (feat * ar).sum(-1)
        e = el[src] + er[dst]
        e = np.where(e > 0, e, NEG_SLOPE * e).astype(np.float32)
        emax = np.full((n, Hh), -np.inf, np.float32)
        np.maximum.at(emax, dst, e)
        ex = np.exp(e - emax[dst])
        den = np.zeros((n, Hh), np.float32)
        np.add.at(den, dst, ex)
        alpha = ex / den[dst]
        out = np.zeros((n, Hh, Dd), np.float32)
        np.add.at(out, dst, feat[src] * alpha[..., None])
        return out + b.reshape(1, Hh, Dd)

    h = np.maximum(gat(feats, W1, al1, ar1, b1).reshape(n, HH), 0.0)
    h = np.maximum(gat(h, Wh, alh, arh, bh).mean(1), 0.0)
    h = gat(h, W2, al2, ar2, b2).mean(1)
    m = h.max(1, keepdims=True)
    ls = np.log(np.exp(h - m).sum(1, keepdims=True))
    return (h - m - ls).astype(np.float32)


def _prep_core_inputs(feats, gblk, par, r, nv, ns_pad, common):
    G = ns_pad // 128
    x = np.zeros((IN, ns_pad), np.float16)
    x[:, :nv] = feats[r * nv:(r + 1) * nv].T.astype(np.float16)
    bp = np.zeros(ns_pad * DEG, np.int64)
    bp[:nv * DEG] = gblk[r * nv * DEG:(r + 1) * nv * DEG]
    pp = np.zeros(ns_pad * DEG, np.uint8)
    pp[:nv * DEG] = par[r * nv * DEG:(r + 1) * nv * DEG]
    # edge (node m, slot k): m = g*128 + p.  Gather h covers slots 8h..8h+8;
    # its list position i maps to (p = i%128, j = i//128, k = 8h+j).
    e = bp.reshape(G, 128, DEG)                  # [g, p, k]
    ep = pp.reshape(G, 128, DEG)
    idx = np.zeros((128, G * 128), np.int16)
    sel = np.zeros((128, G * DEG), np.uint8)
    for g in range(G):
        for h in range(2):
            lst = e[g, :, 8 * h:8 * h + 8].astype(
                np.int16).T.reshape(-1)          # i = j*128 + p
            a = lst.reshape(64, 16).T            # [16, 64]
            idx[:, g * 128 + h * 64:g * 128 + (h + 1) * 64] = np.tile(
                a, (8, 1))
        sel[:, g * DEG:(g + 1) * DEG] = ep[g]
    return dict(x0t=x, idx=idx, sel=sel, **common)


def kernel(**inputs) -> np.ndarray:
    global LAST_RESULTS
    feats = np.ascontiguousarray(np.asarray(inputs["features"],
                                            dtype=np.float32))
    src = np.asarray(inputs["src"]).astype(np.int64).ravel()
    dst = np.asarray(inputs["dst"]).astype(np.int64).ravel()
    W1 = np.asarray(inputs["W1"], dtype=np.float32)
    al1 = np.asarray(inputs["al1"], dtype=np.float32)
    ar1 = np.asarray(inputs["ar1"], dtype=np.float32)
    b1 = np.asarray(inputs["b1"], dtype=np.float32)
    Wh = np.asarray(inputs["Wh"], dtype=np.float32)
    alh = np.asarray(inputs["alh"], dtype=np.float32)
    arh = np.asarray(inputs["arh"], dtype=np.float32)
    bh = np.asarray(inputs["bh"], dtype=np.float32)
    W2 = np.asarray(inputs["W2"], dtype=np.float32)
    al2 = np.asarray(inputs["al2"], dtype=np.float32)
    ar2 = np.asarray(inputs["ar2"], dtype=np.float32)
    b2 = np.asarray(inputs["b2"], dtype=np.float32)

    n = feats.shape[0]
    expected_dst = np.repeat(np.arange(N, dtype=np.int64), DEG)
    if (n != N or src.shape[0] != N * DEG
            or not np.array_equal(dst, expected_dst)
            or src.min() < 0 or src.max() >= N):
        return _numpy_fallback(feats, src, dst, W1, al1, ar1, b1,
                               Wh, alh, arh, bh, W2, al2, ar2, b2)

    from concourse.bass_utils import run_bass_kernel_spmd

    G = (NV + 127) // 128
    ns_pad = G * 128  # 6272
    nc = _get_program(NCORES, ns_pad)

    # chunk-major global pair-block ids
    NCH = len(CHG)
    GS0 = np.array([sum(CHG[:c]) for c in range(NCH)])
    B0 = GS0 * 64
    NBLK = np.array(CHG) * 64
    GOFF = 8 * B0
    chunk_of_group = np.zeros(G, np.int64)
    for c in range(NCH):
        chunk_of_group[GS0[c]:GS0[c] + CHG[c]] = c
    r_s = src // NV
    l_s = src % NV
    b_s = l_s // 2
    c_s = chunk_of_group[b_s // 64]
    gblk = GOFF[c_s] + r_s * NBLK[c_s] + (b_s - B0[c_s])
    par = (l_s % 2).astype(np.uint8)

    def bcast(a, w):
        return np.ascontiguousarray(
            np.broadcast_to(a.reshape(1, w), (128, w)).astype(np.float32))

    def ext(W, al, ar):
        Hh, Dd = al.shape
        Wr = W.reshape(W.shape[0], Hh, Dd)
        wal = np.einsum("khd,hd->kh", Wr, al).astype(np.float32)
        war = np.einsum("khd,hd->kh", Wr, ar).astype(np.float32)
        return np.ascontiguousarray(
            np.concatenate([W, wal, war], axis=1).astype(np.float16))

    common = dict(
        w1=ext(W1, al1, ar1), wh=ext(Wh, alh, arh), w2=ext(W2, al2, ar2),
        b1=bcast(b1, HH), bh=bcast(bh, HH), b2=bcast(b2, OUT),
    )
    in_maps = [
        _prep_core_inputs(feats, gblk, par, r, NV, ns_pad, common)
        for r in range(NCORES)
    ]

    trace = bool(int(os.environ.get("GAT_TRACE", "0")))
    LAST_RESULTS = run_bass_kernel_spmd(
        nc, in_maps, list(range(NCORES)), trace=trace)
    outs = [LAST_RESULTS.results[r]["out"][:NV] for r in range(NCORES)]
    return np.ascontiguousarray(np.concatenate(outs, axis=0),
                                dtype=np.float32)
